# revision 1
# baseline (speedup 1.0000x reference)
"""DigitCaps dynamic-routing kernel for 8 Trainium2 NeuronCores.

Strategy: data-parallel over batch (32 per core), W replicated. u_hat
([256,1152,10,16], 189 MB) is never materialized: each routing iteration folds
the routing coefficients c_ij into a bf16 copy of W ("Wc") and computes
s[b,(c,d)] as one 72-chunk accumulated PE matmul with contraction over (i, r)
(chunks round-robin over 3 PSUM banks). The per-iteration agreement statistic
A[r,c] = mean_b <u_hat, v> is computed without u_hat as
A = sum_(i,d) W ⊙ (p^T v), with 72 bank-alternating p^T v PE matmuls, a DVE
elementwise multiply and a two-step DVE reduce; A is then all-reduced across
the 8 cores (b_ij is shared across the full batch). The third iteration's b_ij
update is dead code in the reference, so only 2 all-reduces run. All matmul
operands are bf16 (fp32 PSUM accumulation), stats and squash in fp32; the
squash sqrt uses a bit-magic rsqrt + 2 Newton steps on DVE so the ACT engine
only ever loads the Exp table. A dummy AllReduce issued during the load phase
absorbs the one-time collective-firmware warmup.

p^T is prepared host-side as a second DMA input (pure index permutation),
eliminating 72 PE transpose matmuls and their PSUM drains.

Measured on trn2.8x1: ~184-196 us HW exec, max rel err 6.4e-3 vs the fp32
reference (bf16 operand quantization, validated against a numpy bit-accuracy
model of the same quantization points).
"""
import numpy as np
from contextlib import ExitStack

import concourse.bass as bass
from concourse import bacc
import concourse.tile as tile
from concourse import mybir
from concourse.bass_utils import run_bass_kernel_spmd

N_CORES = 8
B_FULL, R, C, D, I = 256, 1152, 10, 16, 8
B = B_FULL // N_CORES          # 32 batch per core
G = R // 128                   # 9 chunks of 128 routes
RI = R * I                     # 9216
CD = C * D                     # 160
CDI = C * D * I                # 1280
NUM_IT = 3

FP32 = mybir.dt.float32
BF16 = mybir.dt.bfloat16
ALU = mybir.AluOpType
AX = mybir.AxisListType
AF = mybir.ActivationFunctionType

IPOS = {i: i for i in range(I)}


def _build_body(ctx: ExitStack, tc: "tile.TileContext", p_dram, pt_dram, w_dram,
                v_dram):
    nc = tc.nc

    consts = ctx.enter_context(tc.tile_pool(name="consts", bufs=1))
    pers = ctx.enter_context(tc.tile_pool(name="pers", bufs=1))
    small = ctx.enter_context(tc.tile_pool(name="small", bufs=2))
    dram = ctx.enter_context(tc.tile_pool(name="dram", bufs=2, space="DRAM"))
    ps_s = ctx.enter_context(tc.tile_pool(name="ps_s", bufs=1, space="PSUM"))
    ps_y = ctx.enter_context(tc.tile_pool(name="ps_y", bufs=3, space="PSUM"))

    pb = pers.tile([B, RI], BF16, tag="pb")
    pb_v = pb[:].rearrange("b (g j i) -> b g j i", g=G, j=128, i=I)
    # [(r,i) chunk, b] blocks; 96 cols of tail padding let every s-matmul
    # load a full 128-wide stationary operand (FWL) -- the extra columns
    # produce garbage on PSUM partitions 32..127, which are never read
    pT = pers.tile([128, I * G * B + 96], BF16, tag="pT")
    # W bf16, free order (k, c, d) with k the IPOS-interleaved i position
    wre = [pers.tile([128, CDI], BF16, tag=f"wre{g}", name=f"wre{g}")
           for g in range(G)]

    PCH = RI // G                                     # 1024 p-elems per chunk

    # dummy AllReduce during the load phase: absorbs the one-time collective
    # firmware/credit warmup so the first real AllReduce behaves like the
    # second (~15us cheaper window, measured asymmetry AR#1 vs AR#2)
    warm_sb = consts.tile([128, 128], FP32, tag="warm_sb")
    nc.gpsimd.memset(warm_sb[:], 0.0)
    warm_in = dram.tile([128, 128], FP32, tag="warm_in")
    warm_out = dram.tile([128, 128], FP32, tag="warm_out", addr_space="Shared")
    nc.gpsimd.dma_start(warm_in[:], warm_sb[:])
    nc.gpsimd.collective_compute(
        "AllReduce", ALU.add,
        replica_groups=[list(range(N_CORES))],
        ins=[warm_in[:].opt()],
        outs=[warm_out[:].opt()],
    )

    with ExitStack() as s0:
        # fp32 staging pools: released after stage 0
        pstg = s0.enter_context(tc.tile_pool(name="pstg", bufs=3))
        w32p = s0.enter_context(tc.tile_pool(name="w32p", bufs=G))

        # p natural layout (for the Y matmuls), chunked per g
        for g in range(G):
            p32 = pstg.tile([B, PCH], FP32, tag="p32")
            nc.gpsimd.dma_start(p32[:], p_dram[:, g * PCH:(g + 1) * PCH])
            dst = pb[:, g * PCH:(g + 1) * PCH]
            if g % 2 == 0:
                nc.vector.tensor_copy(dst, p32[:])
            else:
                nc.scalar.copy(dst, p32[:])
        # pre-transposed p (host-prepared layout), chunked: DMA + bf16 cast
        TCH = I * G * B // 3
        for h in range(3):
            pt32 = pstg.tile([128, TCH], FP32, tag="pt32")
            nc.gpsimd.dma_start(pt32[:], pt_dram[:, h * TCH:(h + 1) * TCH])
            if h % 2 == 0:
                nc.scalar.copy(pT[:, h * TCH:(h + 1) * TCH], pt32[:])
            else:
                nc.vector.tensor_copy(pT[:, h * TCH:(h + 1) * TCH], pt32[:])

        # W pipeline: host ships W already in (i,c,d) order -> contiguous cast
        for g in range(G):
            w32 = w32p.tile([128, CDI], FP32)
            nc.sync.dma_start(w32[:], w_dram[128 * g:128 * (g + 1), :])
            if g % 2 == 0:
                nc.vector.tensor_copy(wre[g][:], w32[:])
            else:
                nc.scalar.copy(wre[g][:], w32[:])

    wcp = ctx.enter_context(tc.tile_pool(name="wcp", bufs=1))
    work = ctx.enter_context(tc.tile_pool(name="work", bufs=2))

    magic_t = consts.tile([B, C], mybir.dt.int32, tag="magic_t")
    nc.gpsimd.memset(magic_t[:], 0x5F3759DF)

    # routing logits, [128, (g c)] layout
    bij = pers.tile([128, G * C], FP32, tag="bij")
    nc.gpsimd.memset(bij[:], 0.0)
    bij_v = bij[:].rearrange("p (g c) -> p g c", g=G, c=C)

    def w_slice(t, i):
        """[128, (c,d)] contiguous view of a W tile for true i index."""
        k = IPOS[i]
        return t[:, k * CD:(k + 1) * CD]

    # ---------------- routing iterations ----------------
    for t in range(NUM_IT):
        last = t == NUM_IT - 1
        if t == 0:
            wc = wre                      # c_ij uniform: fold 0.1 into squash
            sqrt_e = 0.1
            e_scale = 0.01
        else:
            sqrt_e = 1.0
            e_scale = 1.0
            # softmax over c of bij -> cbb (bf16)
            mx = small.tile([128, G], FP32, tag="mx")
            nc.vector.tensor_reduce(mx[:], bij_v, axis=AX.X, op=ALU.max)
            eb = small.tile([128, G * C], FP32, tag="eb")
            eb_v = eb[:].rearrange("p (g c) -> p g c", g=G, c=C)
            mxb = mx[:].unsqueeze(2).broadcast_to([128, G, C])
            nc.vector.tensor_tensor(eb_v, bij_v, mxb, op=ALU.subtract)
            nc.scalar.activation(eb[:], eb[:], AF.Exp)
            sm = small.tile([128, G], FP32, tag="sm")
            nc.vector.tensor_reduce(sm[:], eb_v, axis=AX.X, op=ALU.add)
            rc = small.tile([128, G], FP32, tag="rc")
            nc.vector.reciprocal(rc[:], sm[:])
            cbb = small.tile([128, G * C], BF16, tag="cbb")
            cbb_v = cbb[:].rearrange("p (g c) -> p g c", g=G, c=C)
            rcb = rc[:].unsqueeze(2).broadcast_to([128, G, C])
            nc.vector.tensor_tensor(cbb_v, eb_v, rcb, op=ALU.mult)

            # Wc[g] = wre[g] * c  (broadcast over i-position and d), on DVE
            wc = [wcp.tile([128, CDI], BF16, tag=f"wc{g}", name=f"wc{g}_{t}")
                  for g in range(G)]
            for g in range(G):
                w4 = wre[g][:].rearrange("p (k c d) -> p k c d", k=I, c=C, d=D)
                o4 = wc[g][:].rearrange("p (k c d) -> p k c d", k=I, c=C, d=D)
                cb4 = cbb[:, g * C:(g + 1) * C].unsqueeze(1).unsqueeze(3) \
                    .broadcast_to([128, I, C, D])
                nc.vector.tensor_tensor(o4, w4, cb4, op=ALU.mult)

        NSB = 3
        s_parts = [ps_s.tile([128, CD], FP32, tag=f"s_ps{q}",
                             name=f"s_ps{q}_{t}") for q in range(NSB)]
        n_tot = G * I
        n_mm = 0
        for g in range(G):
            for i in range(I):
                k = i * G + g
                nc.tensor.matmul(
                    s_parts[n_mm % NSB][:],
                    pT[:, k * B:k * B + 128],
                    w_slice(wc[g], i),
                    start=(n_mm < NSB),
                    stop=(n_mm >= n_tot - NSB),
                )
                n_mm += 1

        # squash: v = s_eff * sqrt(sq)/(1+sq), sq = |s_eff|^2, s_eff = sqrt_e*s
        s_sb = small.tile([B, CD], FP32, tag="s_sb")
        nc.scalar.copy(s_sb[:], s_parts[0][0:B, :])
        for q in range(1, NSB):
            nc.vector.scalar_tensor_tensor(
                out=s_sb[:], in0=s_parts[q][0:B, :], scalar=1.0, op0=ALU.mult,
                in1=s_sb[:], op1=ALU.add)
        s2 = small.tile([B, CD], FP32, tag="s2")
        nc.vector.tensor_tensor(s2[:], s_sb[:], s_sb[:], op=ALU.mult)
        sq = small.tile([B, C], FP32, tag="sq")
        nc.vector.tensor_reduce(sq[:],
                                s2[:].rearrange("b (c d) -> b c d", c=C, d=D),
                                axis=AX.X, op=ALU.add)
        # r1 = sqrt(e*sq) via bit-magic rsqrt + 2 Newton steps (all DVE)
        m = small.tile([B, C], FP32, tag="m")
        nc.vector.tensor_scalar_mul(m[:], sq[:], e_scale)
        h32 = small.tile([B, C], mybir.dt.int32, tag="h32")
        nc.vector.tensor_scalar(h32[:], m[:].bitcast(mybir.dt.int32), 1, None,
                                op0=ALU.logical_shift_right)
        y0i = small.tile([B, C], mybir.dt.int32, tag="y0i")
        nc.vector.tensor_tensor(y0i[:], magic_t[:], h32[:], op=ALU.subtract)
        y = y0i[:].bitcast(FP32)
        ya = small.tile([B, C], FP32, tag="ya")
        yb = small.tile([B, C], FP32, tag="yb")
        for it in range(2):
            nc.vector.tensor_tensor(ya[:], y, y, op=ALU.mult)
            nc.vector.tensor_tensor(yb[:], ya[:], m[:], op=ALU.mult)
            nc.vector.tensor_scalar(yb[:], yb[:], -0.5, 1.5, op0=ALU.mult,
                                    op1=ALU.add)
            yn = small.tile([B, C], FP32, tag=f"yn{it}", name=f"yn{it}_{t}")
            nc.vector.tensor_tensor(yn[:], y, yb[:], op=ALU.mult)
            y = yn[:]
        r1 = small.tile([B, C], FP32, tag="r1")
        nc.vector.tensor_tensor(r1[:], m[:], y, op=ALU.mult)
        den = small.tile([B, C], FP32, tag="den")
        nc.vector.tensor_scalar(den[:], sq[:], e_scale, 1.0, op0=ALU.mult,
                                op1=ALU.add)
        rec = small.tile([B, C], FP32, tag="rec")
        nc.vector.reciprocal(rec[:], den[:])
        fac = small.tile([B, C], FP32, tag="fac")
        nc.vector.tensor_tensor(fac[:], r1[:], rec[:], op=ALU.mult)

        v32 = small.tile([B, CD], FP32, tag="v32")
        fb = fac[:].unsqueeze(2).broadcast_to([B, C, D])
        nc.vector.scalar_tensor_tensor(
            out=v32[:].rearrange("b (c d) -> b c d", c=C, d=D),
            in0=s_sb[:].rearrange("b (c d) -> b c d", c=C, d=D),
            scalar=sqrt_e, op0=ALU.mult, in1=fb, op1=ALU.mult)

        if last:
            nc.sync.dma_start(v_dram[:, :], v32[:])
            continue

        # ---- agreement stats: A[r, c] = sum_{i,d} W ⊙ (p^T v), AllReduce ----
        vb = small.tile([B, CD], BF16, tag="vb")
        nc.scalar.copy(vb[:], v32[:])

        Apart = pers.tile([128, G * C], FP32, tag="Apart", name=f"Apart{t}")
        cc_in = dram.tile([128, G * C], FP32, tag="cc_in")
        for g in range(G):
            y_sb = work.tile([128, CDI], BF16, tag="y_sb",
                             name=f"y_sb{g}_{t}")
            y_tiles = [ps_y.tile([128, 2 * CD], FP32, tag="y_ps",
                                 name=f"y_ps{g}_{t}_{ip}")
                       for ip in range(I // 2)]
            for h in range(2):
                for ip in range(I // 2):
                    i = 2 * ip + h
                    nc.tensor.matmul(y_tiles[ip][:, h * CD:(h + 1) * CD],
                                     pb_v[:, g, :, i], vb[:],
                                     start=True, stop=True)
            for ip in range(I // 2):
                nc.scalar.copy(
                    y_sb[:, 2 * ip * CD:(2 * ip + 2) * CD], y_tiles[ip][:])
            prod = work.tile([128, CDI], BF16, tag="prod",
                             name=f"prod{g}_{t}")
            nc.vector.tensor_tensor(prod[:], wre[g][:], y_sb[:], op=ALU.mult)
            # A_g = sum over (d, i): contiguous d-reduce, then tiny i-reduce
            pg1 = small.tile([128, I * C], FP32, tag="pg1")
            nc.vector.tensor_reduce(
                pg1[:],
                prod[:].rearrange("p (k c d) -> p k c d", k=I, c=C, d=D),
                axis=AX.X, op=ALU.add)
            nc.vector.tensor_reduce(
                Apart[:, g * C:(g + 1) * C],
                pg1[:].rearrange("p (k c) -> p c k", k=I, c=C),
                axis=AX.X, op=ALU.add)
            # stage this chunk's collective input immediately
            nc.sync.dma_start(cc_in[:, g * C:(g + 1) * C],
                              Apart[:, g * C:(g + 1) * C])

        cc_out = dram.tile([128, G * C], FP32, tag="cc_out",
                           addr_space="Shared")
        nc.gpsimd.collective_compute(
            "AllReduce", ALU.add,
            replica_groups=[list(range(N_CORES))],
            ins=[cc_in[:].opt()],
            outs=[cc_out[:].opt()],
        )
        acc = small.tile([128, G * C], FP32, tag="acc")
        nc.sync.dma_start(acc[:], cc_out[:])
        nc.vector.scalar_tensor_tensor(
            out=bij[:], in0=acc[:], scalar=1.0 / B_FULL, op0=ALU.mult,
            in1=bij[:], op1=ALU.add)


_CACHED = None


def _build():
    global _CACHED
    if _CACHED is not None:
        return _CACHED
    nc = bacc.Bacc("TRN2", target_bir_lowering=False, debug=False,
                   num_devices=N_CORES)
    p_dram = nc.dram_tensor("p_in", [B, RI], FP32, kind="ExternalInput").ap()
    pt_dram = nc.dram_tensor("pt_in", [128, I * G * B], FP32,
                             kind="ExternalInput").ap()
    w_dram = nc.dram_tensor("w_in", [R, CDI], FP32, kind="ExternalInput").ap()
    v_dram = nc.dram_tensor("v_out", [B, CD], FP32, kind="ExternalOutput").ap()
    with tile.TileContext(nc) as tc:
        with ExitStack() as ctx:
            _build_body(ctx, tc, p_dram, pt_dram, w_dram, v_dram)
    nc.finalize()
    _CACHED = nc
    return nc


def kernel(prim_caps: np.ndarray, W: np.ndarray, _trace: bool = False):
    assert prim_caps.shape == (B_FULL, R, I) and W.shape == (1, R, C, D, I)
    nc = _build()
    p_flat = np.ascontiguousarray(prim_caps.reshape(B_FULL, RI).astype(np.float32))
    # pre-shuffle W to (r, i, c, d) so the on-device bf16 cast is contiguous
    w_flat = np.ascontiguousarray(
        W.reshape(R, C, D, I).transpose(0, 3, 1, 2).reshape(R, CDI)
        .astype(np.float32))
    in_maps = []
    for k in range(N_CORES):
        pk = p_flat[k * B:(k + 1) * B]                     # [B, RI]
        # pT[j, (i, g, b)] = p[b, (g*128 + j)*8 + i]
        ptk = np.ascontiguousarray(
            pk.reshape(B, G, 128, I).transpose(2, 3, 1, 0).reshape(128, -1))
        in_maps.append({"p_in": np.ascontiguousarray(pk),
                        "pt_in": ptk, "w_in": w_flat})
    res = run_bass_kernel_spmd(nc, in_maps, core_ids=list(range(N_CORES)),
                               trace=_trace)
    out = np.concatenate(
        [res.results[k]["v_out"].reshape(B, C, D, 1) for k in range(N_CORES)],
        axis=0)
    if _trace:
        return out, res
    return out



# revision 3
# speedup vs baseline: 1.0416x; 1.0416x over previous
"""DigitCaps dynamic-routing kernel for 8 Trainium2 NeuronCores (v2).

Data-parallel over batch (32/core), W replicated, b_ij kept globally
consistent via one fp32 [1152,10] AllReduce per routing update (validated:
per-core batch means diverge far past tolerance, so the collectives are
semantically required).

v2 structural changes over the 195us baseline:
- All inputs ship as bf16 in their final on-chip layouts (host-side cast +
  permutation): no staging pools, no on-device fp32->bf16 casts, ~4.2 MB of
  input DMA instead of 8.3 MB. t=0 compute overlaps the input DMA and hides
  entirely under the one-time ~44us collective-init barrier, which is pulled
  to the front by a tiny warmup AllReduce issued as the first instruction.
- s accumulates 72 chunked matmuls into a single PSUM bank (start/stop
  flags); the 3-bank merge is gone. pT ships with each 32-batch column block
  replicated 3x, so s lands already replicated on PSUM partitions 0..95 and
  the squash chain runs once at [96,*]; its final scalar_tensor_tensor ops
  write the three partition-diagonal blocks of the block-diagonal moving
  operand vb3 directly (elementwise engines cannot cross partitions).
- Agreement phase: y = p^T v is computed 3-i-planes-per-matmul against the
  [96,480] block-diagonal vb3 (27 matmuls/iter instead of 72), drained
  PSUM->SBUF by the scalar engine, multiplied with W on the Pool engine
  (gpsimd) scattered into (c,k,d) order, and reduced in a single DVE
  tensor_reduce per route chunk. Engines pipeline per chunk.
- Softmax skips the max-subtraction (|b_ij| <= 2.8 measured, exp is safe),
  squash uses bit-magic rsqrt + 1 Newton step (rel_max 6.0e-3 in the numpy
  bit-model of exactly these quantization points).
"""
import numpy as np
import ml_dtypes
from contextlib import ExitStack

import concourse.bass as bass
from concourse import bacc
import concourse.tile as tile
from concourse import mybir
from concourse.bass_utils import run_bass_kernel_spmd

N_CORES = 8
B_FULL, R, C, D, I = 256, 1152, 10, 16, 8
B = B_FULL // N_CORES          # 32 batch per core
G = R // 128                   # 9 chunks of 128 routes
CD = C * D                     # 160
CDI = C * D * I                # 1280
NUM_IT = 3
PT_W = 72 * 96 + 32            # 6944: 72 (i,g) blocks of 3x-replicated batch
P3_W = G * 3 * 128             # 3456

FP32 = mybir.dt.float32
BF16 = mybir.dt.bfloat16
ALU = mybir.AluOpType
AX = mybir.AxisListType
AF = mybir.ActivationFunctionType


def _build_body(ctx: ExitStack, tc: "tile.TileContext", pt_dram, p3_dram,
                w_dram, v_dram):
    nc = tc.nc

    consts = ctx.enter_context(tc.tile_pool(name="consts", bufs=1))
    pers = ctx.enter_context(tc.tile_pool(name="pers", bufs=1))
    small = ctx.enter_context(tc.tile_pool(name="small", bufs=2))
    work = ctx.enter_context(tc.tile_pool(name="work", bufs=2))
    wcp = ctx.enter_context(tc.tile_pool(name="wcp", bufs=1))
    dram = ctx.enter_context(tc.tile_pool(name="dram", bufs=2, space="DRAM"))
    ps_s = ctx.enter_context(tc.tile_pool(name="ps_s", bufs=2, space="PSUM"))
    ps_y = ctx.enter_context(tc.tile_pool(name="ps_y", bufs=2, space="PSUM"))

    # warmup AllReduce, first instructions on the gpsimd queue: pulls the
    # one-time collective-firmware barrier (~44us) to the very front so it
    # overlaps the load phase + t=0 compute instead of gating the first real
    # AllReduce.
    warm_sb = consts.tile([128, 16], FP32, tag="warm_sb")
    nc.gpsimd.memset(warm_sb[:], 0.0)
    warm_in = dram.tile([128, 16], FP32, tag="warm_in")
    warm_out = dram.tile([128, 16], FP32, tag="warm_out", addr_space="Shared")
    nc.gpsimd.dma_start(warm_in[:], warm_sb[:])
    nc.gpsimd.collective_compute(
        "AllReduce", ALU.add,
        replica_groups=[list(range(N_CORES))],
        ins=[warm_in[:].opt()],
        outs=[warm_out[:].opt()],
    )

    # ---------------- input tiles, DMA'd bf16 in final layout ----------------
    # pT[j, k*96 + rep*32 + b] = p[b, (g*128+j)*8 + i], k = i*G+g, rep 0..2
    pT = pers.tile([128, PT_W], BF16, tag="pT")
    # p3[(i_rel*32+b), (g*3+grp)*128 + j] = p[b, r, grp*3+i_rel] (i=8 zeros)
    p3 = pers.tile([96, P3_W], BF16, tag="p3")
    # W bf16, (r -> partition j within chunk g, free (i, c, d))
    wre = [pers.tile([128, CDI], BF16, tag=f"wre{g}", name=f"wre{g}")
           for g in range(G)]

    # spread triggers over three queues; W chunk order matches s-matmul use
    for g in (0, 1, 2, 3, 6):
        nc.sync.dma_start(wre[g][:], w_dram[128 * g:128 * (g + 1), :])
    H = PT_W // 2
    nc.scalar.dma_start(pT[:, :H], pt_dram[:, :H])
    nc.scalar.dma_start(pT[:, H:], pt_dram[:, H:])
    for g in (4, 5, 7, 8):
        nc.scalar.dma_start(wre[g][:], w_dram[128 * g:128 * (g + 1), :])
    H3 = P3_W // 2
    nc.gpsimd.dma_start(p3[:, :H3], p3_dram[:, :H3])
    nc.gpsimd.dma_start(p3[:, H3:], p3_dram[:, H3:])

    magic_t = consts.tile([96, C], mybir.dt.int32, tag="magic_t")
    nc.gpsimd.memset(magic_t[:], 0x5F3759DF)
    # routing logits, [128, (g c)] layout
    bij = pers.tile([128, G * C], FP32, tag="bij")
    nc.gpsimd.memset(bij[:], 0.0)
    bij_v = bij[:].rearrange("p (g c) -> p g c", g=G, c=C)
    # block-diagonal moving operand for the agreement matmuls; off-diagonal
    # stays zero forever, diagonal blocks are rewritten by the squash
    vb3 = pers.tile([96, 3 * CD], BF16, tag="vb3")
    nc.gpsimd.memset(vb3[:], 0.0)

    cc_out_prev = None

    # ---------------- routing iterations ----------------
    for t in range(NUM_IT):
        last = t == NUM_IT - 1
        if t == 0:
            wc = wre                      # c_ij uniform: fold 0.1 into squash
            e_scale = 0.01
        else:
            e_scale = 1.0
            # b update from the previous iteration's AllReduce
            acc = small.tile([128, G * C], FP32, tag="acc", name=f"acc{t}")
            nc.sync.dma_start(acc[:], cc_out_prev[:])
            nc.vector.scalar_tensor_tensor(
                out=bij[:], in0=acc[:], scalar=1.0 / B_FULL, op0=ALU.mult,
                in1=bij[:], op1=ALU.add)
            # softmax over c, no max-subtract (|b| <= ~3)
            eb = small.tile([128, G * C], FP32, tag="eb", name=f"eb{t}")
            nc.scalar.activation(eb[:], bij[:], AF.Exp)
            eb_v = eb[:].rearrange("p (g c) -> p g c", g=G, c=C)
            sm = small.tile([128, G], FP32, tag="sm", name=f"sm{t}")
            nc.vector.tensor_reduce(sm[:], eb_v, axis=AX.X, op=ALU.add)
            rc = small.tile([128, G], FP32, tag="rc", name=f"rc{t}")
            nc.vector.reciprocal(rc[:], sm[:])
            cbb = small.tile([128, G * C], BF16, tag="cbb", name=f"cbb{t}")
            cbb_v = cbb[:].rearrange("p (g c) -> p g c", g=G, c=C)
            rcb = rc[:].unsqueeze(2).broadcast_to([128, G, C])
            nc.vector.tensor_tensor(cbb_v, eb_v, rcb, op=ALU.mult)

            # Wc[g] = wre[g] * c, split DVE / Pool
            wc = [wcp.tile([128, CDI], BF16, tag=f"wc{g}", name=f"wc{g}_{t}")
                  for g in range(G)]
            for g in range(G):
                w4 = wre[g][:].rearrange("p (k c d) -> p k c d", k=I, c=C, d=D)
                o4 = wc[g][:].rearrange("p (k c d) -> p k c d", k=I, c=C, d=D)
                cb4 = cbb[:, g * C:(g + 1) * C].unsqueeze(1).unsqueeze(3) \
                    .broadcast_to([128, I, C, D])
                eng = nc.vector if g % 2 == 0 else nc.gpsimd
                eng.tensor_tensor(o4, w4, cb4, op=ALU.mult)

        # s[b,(c,d)] = sum_{r,i} p * Wc: 72 matmuls, one accumulating PSUM
        # bank; output replicated on partition blocks 0:32/32:64/64:96
        s_ps = ps_s.tile([128, CD], FP32, tag="s_ps", name=f"s_ps_{t}")
        n_tot = G * I
        n_mm = 0
        for g in range(G):
            for i in range(I):
                k = i * G + g
                nc.tensor.matmul(
                    s_ps[:],
                    pT[:, k * 96:k * 96 + 128],
                    wc[g][:, i * CD:(i + 1) * CD],
                    start=(n_mm == 0),
                    stop=(n_mm == n_tot - 1),
                )
                n_mm += 1

        # squash at [96, *]: v = s * e*sqrt(sq)/(1+e*sq), sq = sum_d s^2
        s2 = small.tile([96, CD], FP32, tag="s2", name=f"s2_{t}")
        nc.scalar.activation(s2[:], s_ps[0:96, :], AF.Square)
        sq = small.tile([96, C], FP32, tag="sq", name=f"sq_{t}")
        nc.vector.tensor_reduce(sq[:],
                                s2[:].rearrange("b (c d) -> b c d", c=C, d=D),
                                axis=AX.X, op=ALU.add)
        h32 = small.tile([96, C], mybir.dt.int32, tag="h32", name=f"h32_{t}")
        nc.vector.tensor_scalar(h32[:], sq[:].bitcast(mybir.dt.int32), 1,
                                None, op0=ALU.logical_shift_right)
        y0i = small.tile([96, C], mybir.dt.int32, tag="y0i", name=f"y0i_{t}")
        nc.vector.tensor_tensor(y0i[:], magic_t[:], h32[:], op=ALU.subtract)
        y = y0i[:].bitcast(FP32)
        ya = small.tile([96, C], FP32, tag="ya", name=f"ya_{t}")
        yb = small.tile([96, C], FP32, tag="yb", name=f"yb_{t}")
        nc.vector.tensor_tensor(ya[:], y, y, op=ALU.mult)
        nc.vector.tensor_tensor(yb[:], ya[:], sq[:], op=ALU.mult)
        nc.vector.tensor_scalar(yb[:], yb[:], -0.5, 1.5, op0=ALU.mult,
                                op1=ALU.add)
        yn = small.tile([96, C], FP32, tag="yn", name=f"yn_{t}")
        nc.vector.tensor_tensor(yn[:], y, yb[:], op=ALU.mult)
        r1 = small.tile([96, C], FP32, tag="r1", name=f"r1_{t}")
        nc.vector.tensor_tensor(r1[:], sq[:], yn[:], op=ALU.mult)
        den = small.tile([96, C], FP32, tag="den", name=f"den_{t}")
        nc.vector.tensor_scalar(den[:], sq[:], e_scale, 1.0, op0=ALU.mult,
                                op1=ALU.add)
        rec = small.tile([96, C], FP32, tag="rec", name=f"rec_{t}")
        nc.vector.reciprocal(rec[:], den[:])
        fac = small.tile([96, C], FP32, tag="fac", name=f"fac_{t}")
        nc.vector.tensor_tensor(fac[:], r1[:], rec[:], op=ALU.mult)

        if last:
            v32 = small.tile([B, CD], FP32, tag="v32")
            fb = fac[0:B, :].unsqueeze(2).broadcast_to([B, C, D])
            nc.vector.scalar_tensor_tensor(
                out=v32[:].rearrange("b (c d) -> b c d", c=C, d=D),
                in0=s_ps[0:B, :].rearrange("b (c d) -> b c d", c=C, d=D),
                scalar=e_scale, op0=ALU.mult, in1=fb, op1=ALU.mult)
            nc.sync.dma_start(v_dram[:, :], v32[:])
            continue

        # diagonal blocks of vb3 (s_ps replication keeps this lane-aligned)
        for rp in range(3):
            pa, pb_ = rp * 32, (rp + 1) * 32
            fb = fac[pa:pb_, :].unsqueeze(2).broadcast_to([32, C, D])
            nc.vector.scalar_tensor_tensor(
                out=vb3[pa:pb_, rp * CD:(rp + 1) * CD]
                    .rearrange("b (c d) -> b c d", c=C, d=D),
                in0=s_ps[pa:pb_, :].rearrange("b (c d) -> b c d", c=C, d=D),
                scalar=e_scale, op0=ALU.mult, in1=fb, op1=ALU.mult)

        # ---- agreement: A[r,c] = sum_{i,d} W . (p^T v), AllReduce ----
        Apart = pers.tile([128, G * C], FP32, tag="Apart", name=f"Apart{t}")
        cc_in = dram.tile([128, G * C], FP32, tag="cc_in", name=f"cc_in{t}")
        for g in range(G):
            y0 = ps_y.tile([128, 3 * CD], FP32, tag="y0", name=f"y0_{g}_{t}")
            y1 = ps_y.tile([128, 3 * CD], FP32, tag="y1", name=f"y1_{g}_{t}")
            y2 = ps_y.tile([128, 2 * CD], FP32, tag="y2", name=f"y2_{g}_{t}")
            c0 = (3 * g) * 128
            nc.tensor.matmul(y0[:], p3[:, c0:c0 + 128], vb3[:],
                             start=True, stop=True)
            nc.tensor.matmul(y1[:], p3[:, c0 + 128:c0 + 256], vb3[:],
                             start=True, stop=True)
            nc.tensor.matmul(y2[:], p3[0:64, c0 + 256:c0 + 384],
                             vb3[0:64, 0:2 * CD], start=True, stop=True)
            y0sb = work.tile([128, 3 * CD], BF16, tag="y0sb",
                             name=f"y0sb{g}_{t}")
            y1sb = work.tile([128, 3 * CD], BF16, tag="y1sb",
                             name=f"y1sb{g}_{t}")
            y2sb = work.tile([128, 2 * CD], BF16, tag="y2sb",
                             name=f"y2sb{g}_{t}")
            nc.scalar.copy(y0sb[:], y0[:])
            nc.scalar.copy(y1sb[:], y1[:])
            nc.scalar.copy(y2sb[:], y2[:])
            # prod in (c, k, d) order so one X-reduce yields A[:, (g c)]
            prod = work.tile([128, CDI], BF16, tag="prod",
                             name=f"prod{g}_{t}")
            pv = prod[:].rearrange("p (c k d) -> p k c d", c=C, k=I, d=D)
            wv = wre[g][:].rearrange("p (k c d) -> p k c d", k=I, c=C, d=D)
            nc.gpsimd.tensor_tensor(
                pv[:, 0:3],
                wv[:, 0:3],
                y0sb[:].rearrange("p (k c d) -> p k c d", k=3, c=C, d=D),
                op=ALU.mult)
            nc.gpsimd.tensor_tensor(
                pv[:, 3:6],
                wv[:, 3:6],
                y1sb[:].rearrange("p (k c d) -> p k c d", k=3, c=C, d=D),
                op=ALU.mult)
            nc.gpsimd.tensor_tensor(
                pv[:, 6:8],
                wv[:, 6:8],
                y2sb[:].rearrange("p (k c d) -> p k c d", k=2, c=C, d=D),
                op=ALU.mult)
            nc.vector.tensor_reduce(
                Apart[:, g * C:(g + 1) * C],
                prod[:].rearrange("p (c x) -> p c x", c=C, x=I * D),
                axis=AX.X, op=ALU.add)
            nc.sync.dma_start(cc_in[:, g * C:(g + 1) * C],
                              Apart[:, g * C:(g + 1) * C])

        cc_out_prev = dram.tile([128, G * C], FP32, tag="cc_out",
                                name=f"cc_out{t}", addr_space="Shared")
        nc.gpsimd.collective_compute(
            "AllReduce", ALU.add,
            replica_groups=[list(range(N_CORES))],
            ins=[cc_in[:].opt()],
            outs=[cc_out_prev[:].opt()],
        )


_CACHED = None


def _build():
    global _CACHED
    if _CACHED is not None:
        return _CACHED
    nc = bacc.Bacc("TRN2", target_bir_lowering=False, debug=False,
                   num_devices=N_CORES)
    pt_dram = nc.dram_tensor("pt_in", [128, PT_W], BF16,
                             kind="ExternalInput").ap()
    p3_dram = nc.dram_tensor("p3_in", [96, P3_W], BF16,
                             kind="ExternalInput").ap()
    w_dram = nc.dram_tensor("w_in", [R, CDI], BF16, kind="ExternalInput").ap()
    v_dram = nc.dram_tensor("v_out", [B, CD], FP32, kind="ExternalOutput").ap()
    with tile.TileContext(nc) as tc:
        with ExitStack() as ctx:
            _build_body(ctx, tc, pt_dram, p3_dram, w_dram, v_dram)
    nc.finalize()
    _CACHED = nc
    return nc


def kernel(prim_caps: np.ndarray, W: np.ndarray, _trace: bool = False):
    assert prim_caps.shape == (B_FULL, R, I) and W.shape == (1, R, C, D, I)
    nc = _build()
    bf16 = ml_dtypes.bfloat16
    # W -> (r, i, c, d), bf16
    w_flat = np.ascontiguousarray(
        W.reshape(R, C, D, I).transpose(0, 3, 1, 2).reshape(R, CDI)
        .astype(bf16))
    p32 = prim_caps.astype(np.float32)
    in_maps = []
    for k in range(N_CORES):
        pk = p32[k * B:(k + 1) * B]                       # [B, R*I]-ish
        pk4 = pk.reshape(B, G, 128, I)
        # pT[j, (i,g) block: b replicated 3x], 32 zero pad cols at the end
        ptk = np.zeros((128, PT_W), np.float32)
        ptk[:, :72 * 96] = np.broadcast_to(
            pk4.transpose(2, 3, 1, 0)[:, :, :, None, :],
            (128, I, G, 3, B)).reshape(128, 72 * 96)
        # p3[(i_rel, b), (g, grp, j)], i padded to 9 with zeros
        p9 = np.zeros((B, G, 128, 9), np.float32)
        p9[..., :I] = pk4
        p3k = p9.reshape(B, G, 128, 3, 3).transpose(4, 0, 1, 3, 2) \
            .reshape(96, P3_W)
        in_maps.append({"pt_in": ptk.astype(bf16),
                        "p3_in": np.ascontiguousarray(p3k.astype(bf16)),
                        "w_in": w_flat})
    res = run_bass_kernel_spmd(nc, in_maps, core_ids=list(range(N_CORES)),
                               trace=_trace)
    out = np.concatenate(
        [res.results[k]["v_out"].reshape(B, C, D, 1) for k in range(N_CORES)],
        axis=0)
    if _trace:
        return out, res
    return out


# revision 4
# speedup vs baseline: 1.1613x; 1.1149x over previous
"""DigitCaps dynamic-routing kernel for 8 Trainium2 NeuronCores (v3).

Data-parallel over batch (32/core), W replicated, b_ij kept globally
consistent via fp32 AllReduces of the [1152,10] agreement tensor (validated:
per-core batch means diverge far past tolerance, so the collectives are
semantically required).

Structure (measured engine rates drive the assignment):
- All inputs ship as bf16 in final on-chip layouts (host-side cast +
  permutation): no staging, no on-device casts, ~4.2 MB input DMA. t=0
  compute hides under the fixed ~62us collective-stream init barrier.
- Each routing update's AllReduce is SPLIT into route-chunk halves (g0-4,
  g5-8). The next iteration's softmax/fold/matmuls for the first half start
  as soon as its half lands, hiding most of the second half's latency; the
  t=0 first half also serves as the collective warmup.
- s accumulates 72 chunked matmuls into one PSUM bank; pT ships with each
  32-batch block replicated 3x so s lands replicated on partitions 0..95 and
  the squash writes the three partition-diagonal blocks of the block-diagonal
  agreement operand vb3 with lane-aligned ops.
- Agreement phase per chunk: 3 matmuls against vb3 (3 i-planes each), ACT
  drains PSUM->SBUF bf16, W-multiply split Pool(2)/DVE(1) scattered into
  (c,k,d) order, one DVE X-reduce -> A chunk, staged per-half for the
  collective. Engines pipeline across chunks.
- Wc folds are DVE-only (Pool measured 2.6x slower and concurrent DVE+Pool
  big SBUF ops degrade both ~2.4x); softmax skips max-subtraction
  (|b_ij| <= 2.8 measured); squash uses bit-magic rsqrt + 1 Newton step.
  Numpy bit-model of these quantization points: rel_max 6.0e-3.
"""
import numpy as np
import ml_dtypes
from contextlib import ExitStack

import concourse.bass as bass
from concourse import bacc
import concourse.tile as tile
from concourse import mybir
from concourse.bass_utils import run_bass_kernel_spmd

N_CORES = 8
B_FULL, R, C, D, I = 256, 1152, 10, 16, 8
B = B_FULL // N_CORES          # 32 batch per core
G = R // 128                   # 9 chunks of 128 routes
CD = C * D                     # 160
CDI = C * D * I                # 1280
NUM_IT = 3
PT_W = 72 * 96 + 32            # 6944: 72 (i,g) blocks of 3x-replicated batch
P3_W = G * 3 * 128             # 3456
GA, GB = 5, 4                  # AllReduce halves: g 0..4 and g 5..8

FP32 = mybir.dt.float32
BF16 = mybir.dt.bfloat16
ALU = mybir.AluOpType
AX = mybir.AxisListType
AF = mybir.ActivationFunctionType


def _build_body(ctx: ExitStack, tc: "tile.TileContext", pt_dram, p3_dram,
                w_dram, v_dram):
    nc = tc.nc

    consts = ctx.enter_context(tc.tile_pool(name="consts", bufs=1))
    pers = ctx.enter_context(tc.tile_pool(name="pers", bufs=1))
    small = ctx.enter_context(tc.tile_pool(name="small", bufs=2))
    work = ctx.enter_context(tc.tile_pool(name="work", bufs=2))
    wcp = ctx.enter_context(tc.tile_pool(name="wcp", bufs=1))
    dram = ctx.enter_context(tc.tile_pool(name="dram", bufs=2, space="DRAM"))
    ps_s = ctx.enter_context(tc.tile_pool(name="ps_s", bufs=2, space="PSUM"))
    ps_y = ctx.enter_context(tc.tile_pool(name="ps_y", bufs=2, space="PSUM"))

    # ---------------- input tiles, DMA'd bf16 in final layout ----------------
    # pT[j, k*96 + rep*32 + b] = p[b, (g*128+j)*8 + i], k = i*G+g, rep 0..2
    pT = pers.tile([128, PT_W], BF16, tag="pT")
    # p3[(i_rel*32+b), (g*3+grp)*128 + j] = p[b, r, grp*3+i_rel] (i=8 zeros)
    p3 = pers.tile([96, P3_W], BF16, tag="p3")
    # W bf16, (r -> partition j within chunk g, free (i, c, d))
    wre = [pers.tile([128, CDI], BF16, tag=f"wre{g}", name=f"wre{g}")
           for g in range(G)]

    for g in (0, 1, 2, 3, 6):
        nc.sync.dma_start(wre[g][:], w_dram[128 * g:128 * (g + 1), :])
    H = PT_W // 2
    nc.scalar.dma_start(pT[:, :H], pt_dram[:, :H])
    nc.scalar.dma_start(pT[:, H:], pt_dram[:, H:])
    for g in (4, 5, 7, 8):
        nc.scalar.dma_start(wre[g][:], w_dram[128 * g:128 * (g + 1), :])
    H3 = P3_W // 2
    nc.gpsimd.dma_start(p3[:, :H3], p3_dram[:, :H3])
    nc.gpsimd.dma_start(p3[:, H3:], p3_dram[:, H3:])

    magic_t = consts.tile([96, C], mybir.dt.int32, tag="magic_t")
    nc.gpsimd.memset(magic_t[:], 0x5F3759DF)
    # routing logits, [128, (g c)] layout
    bij = pers.tile([128, G * C], FP32, tag="bij")
    nc.gpsimd.memset(bij[:], 0.0)
    # block-diagonal moving operand for the agreement matmuls; off-diagonal
    # stays zero forever, diagonal blocks are rewritten by the squash
    vb3 = pers.tile([96, 3 * CD], BF16, tag="vb3")
    nc.gpsimd.memset(vb3[:], 0.0)

    cc_out_prev = None             # (half_a, half_b) from previous iteration

    def softmax_fold(t, ha):
        """b update + softmax + Wc fold for one AllReduce half."""
        lo = 0 if ha == 0 else GA * C
        hi = GA * C if ha == 0 else G * C
        ng = GA if ha == 0 else GB
        g0 = 0 if ha == 0 else GA
        w = hi - lo
        acc = small.tile([128, G * C], FP32, tag="acc", name=f"acc{t}_{ha}")
        nc.sync.dma_start(acc[:, lo:hi], cc_out_prev[ha][:])
        nc.vector.scalar_tensor_tensor(
            out=bij[:, lo:hi], in0=acc[:, lo:hi], scalar=1.0 / B_FULL,
            op0=ALU.mult, in1=bij[:, lo:hi], op1=ALU.add)
        eb = small.tile([128, G * C], FP32, tag="eb", name=f"eb{t}_{ha}")
        nc.scalar.activation(eb[:, lo:hi], bij[:, lo:hi], AF.Exp)
        ebv = eb[:, lo:hi].rearrange("p (g c) -> p g c", g=ng, c=C)
        sm = small.tile([128, G], FP32, tag="sm", name=f"sm{t}_{ha}")
        nc.vector.tensor_reduce(sm[:, g0:g0 + ng], ebv, axis=AX.X, op=ALU.add)
        rc = small.tile([128, G], FP32, tag="rc", name=f"rc{t}_{ha}")
        nc.vector.reciprocal(rc[:, g0:g0 + ng], sm[:, g0:g0 + ng])
        cbb = small.tile([128, G * C], BF16, tag="cbb", name=f"cbb{t}_{ha}")
        cbv = cbb[:, lo:hi].rearrange("p (g c) -> p g c", g=ng, c=C)
        rcb = rc[:, g0:g0 + ng].unsqueeze(2).broadcast_to([128, ng, C])
        nc.vector.tensor_tensor(cbv, ebv, rcb, op=ALU.mult)
        # expand c over d once (innermost stride-0), so each fold's in1 has a
        # packed innermost dim (2x DVE mode), broadcast only over k
        cbd = small.tile([128, G * CD], BF16, tag="cbd", name=f"cbd{t}_{ha}")
        nc.vector.tensor_copy(
            cbd[:, lo * D:hi * D].rearrange("p (g c d) -> p g c d",
                                            g=ng, c=C, d=D),
            cbb[:, lo:hi].rearrange("p (g c) -> p g c", g=ng, c=C)
            .unsqueeze(3).broadcast_to([128, ng, C, D]))
        wcs = []
        for g in range(g0, g0 + ng):
            wcg = wcp.tile([128, CDI], BF16, tag=f"wc{g}", name=f"wc{g}_{t}")
            in1 = cbd[:, g * CD:(g + 1) * CD].unsqueeze(1) \
                .broadcast_to([128, I, CD])
            nc.vector.tensor_tensor(
                wcg[:].rearrange("p (k x) -> p k x", k=I, x=CD),
                wre[g][:].rearrange("p (k x) -> p k x", k=I, x=CD),
                in1, op=ALU.mult)
            wcs.append(wcg)
        return wcs

    # ---------------- routing iterations ----------------
    for t in range(NUM_IT):
        last = t == NUM_IT - 1
        if t == 0:
            wc = wre                      # c_ij uniform: fold 0.1 into squash
            e_scale = 0.01
        else:
            e_scale = 1.0

        # s[b,(c,d)] = sum_{r,i} p * Wc: 72 matmuls, one accumulating PSUM
        # bank; output replicated on partition blocks 0:32/32:64/64:96.
        # At t>0 the fold for each half runs as its AllReduce half lands.
        s_ps = ps_s.tile([128, CD], FP32, tag="s_ps", name=f"s_ps_{t}")
        n_tot = G * I
        n_mm = 0
        for ha in range(2):
            gs = range(0, GA) if ha == 0 else range(GA, G)
            if t > 0:
                wch = softmax_fold(t, ha)
                wc = {g: wch[g - (0 if ha == 0 else GA)] for g in gs}
            for g in gs:
                for i in range(I):
                    k = i * G + g
                    nc.tensor.matmul(
                        s_ps[:],
                        pT[:, k * 96:k * 96 + 128],
                        wc[g][:, i * CD:(i + 1) * CD],
                        start=(n_mm == 0),
                        stop=(n_mm == n_tot - 1),
                    )
                    n_mm += 1

        # squash at [96, *]: v = s * e*sqrt(sq)/(1+e*sq), sq = sum_d s^2
        s2 = small.tile([96, CD], FP32, tag="s2", name=f"s2_{t}")
        nc.scalar.activation(s2[:], s_ps[0:96, :], AF.Square)
        sq = small.tile([96, C], FP32, tag="sq", name=f"sq_{t}")
        nc.vector.tensor_reduce(sq[:],
                                s2[:].rearrange("b (c d) -> b c d", c=C, d=D),
                                axis=AX.X, op=ALU.add)
        h32 = small.tile([96, C], mybir.dt.int32, tag="h32", name=f"h32_{t}")
        nc.vector.tensor_scalar(h32[:], sq[:].bitcast(mybir.dt.int32), 1,
                                None, op0=ALU.logical_shift_right)
        y0i = small.tile([96, C], mybir.dt.int32, tag="y0i", name=f"y0i_{t}")
        nc.vector.tensor_tensor(y0i[:], magic_t[:], h32[:], op=ALU.subtract)
        y = y0i[:].bitcast(FP32)
        ya = small.tile([96, C], FP32, tag="ya", name=f"ya_{t}")
        yb = small.tile([96, C], FP32, tag="yb", name=f"yb_{t}")
        nc.vector.tensor_tensor(ya[:], y, y, op=ALU.mult)
        nc.vector.tensor_tensor(yb[:], ya[:], sq[:], op=ALU.mult)
        nc.vector.tensor_scalar(yb[:], yb[:], -0.5, 1.5, op0=ALU.mult,
                                op1=ALU.add)
        yn = small.tile([96, C], FP32, tag="yn", name=f"yn_{t}")
        nc.vector.tensor_tensor(yn[:], y, yb[:], op=ALU.mult)
        r1 = small.tile([96, C], FP32, tag="r1", name=f"r1_{t}")
        nc.vector.tensor_tensor(r1[:], sq[:], yn[:], op=ALU.mult)
        den = small.tile([96, C], FP32, tag="den", name=f"den_{t}")
        nc.vector.tensor_scalar(den[:], sq[:], e_scale, 1.0, op0=ALU.mult,
                                op1=ALU.add)
        rec = small.tile([96, C], FP32, tag="rec", name=f"rec_{t}")
        nc.vector.reciprocal(rec[:], den[:])
        fac = small.tile([96, C], FP32, tag="fac", name=f"fac_{t}")
        nc.vector.tensor_tensor(fac[:], r1[:], rec[:], op=ALU.mult)

        if last:
            v32 = small.tile([B, CD], FP32, tag="v32")
            fb = fac[0:B, :].unsqueeze(2).broadcast_to([B, C, D])
            nc.vector.scalar_tensor_tensor(
                out=v32[:].rearrange("b (c d) -> b c d", c=C, d=D),
                in0=s_ps[0:B, :].rearrange("b (c d) -> b c d", c=C, d=D),
                scalar=e_scale, op0=ALU.mult, in1=fb, op1=ALU.mult)
            nc.sync.dma_start(v_dram[:, :], v32[:])
            continue

        # diagonal blocks of vb3 (s_ps replication keeps this lane-aligned)
        for rp in range(3):
            pa, pb_ = rp * 32, (rp + 1) * 32
            fb = fac[pa:pb_, :].unsqueeze(2).broadcast_to([32, C, D])
            nc.vector.scalar_tensor_tensor(
                out=vb3[pa:pb_, rp * CD:(rp + 1) * CD]
                    .rearrange("b (c d) -> b c d", c=C, d=D),
                in0=s_ps[pa:pb_, :].rearrange("b (c d) -> b c d", c=C, d=D),
                scalar=e_scale, op0=ALU.mult, in1=fb, op1=ALU.mult)

        # ---- agreement: A[r,c] = sum_{i,d} W . (p^T v), split AllReduce ----
        Apart = pers.tile([128, G * C], FP32, tag="Apart", name=f"Apart{t}")
        cc_a = dram.tile([128, GA * C], FP32, tag="cc_a", name=f"cc_a{t}")
        cc_b = dram.tile([128, GB * C], FP32, tag="cc_b", name=f"cc_b{t}")
        for g in range(G):
            y0 = ps_y.tile([128, 3 * CD], FP32, tag="y0", name=f"y0_{g}_{t}")
            y1 = ps_y.tile([128, 3 * CD], FP32, tag="y1", name=f"y1_{g}_{t}")
            y2 = ps_y.tile([128, 2 * CD], FP32, tag="y2", name=f"y2_{g}_{t}")
            c0 = (3 * g) * 128
            nc.tensor.matmul(y0[:], p3[:, c0:c0 + 128], vb3[:],
                             start=True, stop=True)
            nc.tensor.matmul(y1[:], p3[:, c0 + 128:c0 + 256], vb3[:],
                             start=True, stop=True)
            nc.tensor.matmul(y2[:], p3[0:64, c0 + 256:c0 + 384],
                             vb3[0:64, 0:2 * CD], start=True, stop=True)
            y0sb = work.tile([128, 3 * CD], BF16, tag="y0sb",
                             name=f"y0sb{g}_{t}")
            y1sb = work.tile([128, 3 * CD], BF16, tag="y1sb",
                             name=f"y1sb{g}_{t}")
            y2sb = work.tile([128, 2 * CD], BF16, tag="y2sb",
                             name=f"y2sb{g}_{t}")
            nc.scalar.copy(y0sb[:], y0[:])
            nc.scalar.copy(y1sb[:], y1[:])
            nc.scalar.copy(y2sb[:], y2[:])
            # prod in (c, k, d) order so one X-reduce yields A[:, (g c)]
            prod = work.tile([128, CDI], BF16, tag="prod",
                             name=f"prod{g}_{t}")
            pv = prod[:].rearrange("p (c k d) -> p k c d", c=C, k=I, d=D)
            wv = wre[g][:].rearrange("p (k c d) -> p k c d", k=I, c=C, d=D)
            nc.gpsimd.tensor_tensor(
                pv[:, 0:3], wv[:, 0:3],
                y0sb[:].rearrange("p (k c d) -> p k c d", k=3, c=C, d=D),
                op=ALU.mult)
            nc.vector.tensor_tensor(
                pv[:, 3:6], wv[:, 3:6],
                y1sb[:].rearrange("p (k c d) -> p k c d", k=3, c=C, d=D),
                op=ALU.mult)
            nc.gpsimd.tensor_tensor(
                pv[:, 6:8], wv[:, 6:8],
                y2sb[:].rearrange("p (k c d) -> p k c d", k=2, c=C, d=D),
                op=ALU.mult)
            nc.vector.tensor_reduce(
                Apart[:, g * C:(g + 1) * C],
                prod[:].rearrange("p (c x) -> p c x", c=C, x=I * D),
                axis=AX.X, op=ALU.add)
            if g < GA:
                nc.sync.dma_start(cc_a[:, g * C:(g + 1) * C],
                                  Apart[:, g * C:(g + 1) * C])
            else:
                nc.sync.dma_start(cc_b[:, (g - GA) * C:(g - GA + 1) * C],
                                  Apart[:, g * C:(g + 1) * C])
            if g == GA - 1:
                out_a = dram.tile([128, GA * C], FP32, tag="cc_oa",
                                  name=f"cc_oa{t}", addr_space="Shared")
                nc.gpsimd.collective_compute(
                    "AllReduce", ALU.add,
                    replica_groups=[list(range(N_CORES))],
                    ins=[cc_a[:].opt()], outs=[out_a[:].opt()])
        out_b = dram.tile([128, GB * C], FP32, tag="cc_ob",
                          name=f"cc_ob{t}", addr_space="Shared")
        nc.gpsimd.collective_compute(
            "AllReduce", ALU.add,
            replica_groups=[list(range(N_CORES))],
            ins=[cc_b[:].opt()], outs=[out_b[:].opt()])
        cc_out_prev = (out_a, out_b)


_CACHED = None


def _build():
    global _CACHED
    if _CACHED is not None:
        return _CACHED
    nc = bacc.Bacc("TRN2", target_bir_lowering=False, debug=False,
                   num_devices=N_CORES)
    pt_dram = nc.dram_tensor("pt_in", [128, PT_W], BF16,
                             kind="ExternalInput").ap()
    p3_dram = nc.dram_tensor("p3_in", [96, P3_W], BF16,
                             kind="ExternalInput").ap()
    w_dram = nc.dram_tensor("w_in", [R, CDI], BF16, kind="ExternalInput").ap()
    v_dram = nc.dram_tensor("v_out", [B, CD], FP32, kind="ExternalOutput").ap()
    with tile.TileContext(nc) as tc:
        with ExitStack() as ctx:
            _build_body(ctx, tc, pt_dram, p3_dram, w_dram, v_dram)
    nc.finalize()
    _CACHED = nc
    return nc


def kernel(prim_caps: np.ndarray, W: np.ndarray, _trace: bool = False):
    assert prim_caps.shape == (B_FULL, R, I) and W.shape == (1, R, C, D, I)
    nc = _build()
    bf16 = ml_dtypes.bfloat16
    w_flat = np.ascontiguousarray(
        W.reshape(R, C, D, I).transpose(0, 3, 1, 2).reshape(R, CDI)
        .astype(bf16))
    p32 = prim_caps.astype(np.float32)
    in_maps = []
    for k in range(N_CORES):
        pk = p32[k * B:(k + 1) * B]
        pk4 = pk.reshape(B, G, 128, I)
        ptk = np.zeros((128, PT_W), np.float32)
        ptk[:, :72 * 96] = np.broadcast_to(
            pk4.transpose(2, 3, 1, 0)[:, :, :, None, :],
            (128, I, G, 3, B)).reshape(128, 72 * 96)
        p9 = np.zeros((B, G, 128, 9), np.float32)
        p9[..., :I] = pk4
        p3k = p9.reshape(B, G, 128, 3, 3).transpose(4, 0, 1, 3, 2) \
            .reshape(96, P3_W)
        in_maps.append({"pt_in": ptk.astype(bf16),
                        "p3_in": np.ascontiguousarray(p3k.astype(bf16)),
                        "w_in": w_flat})
    res = run_bass_kernel_spmd(nc, in_maps, core_ids=list(range(N_CORES)),
                               trace=_trace)
    out = np.concatenate(
        [res.results[k]["v_out"].reshape(B, C, D, 1) for k in range(N_CORES)],
        axis=0)
    if _trace:
        return out, res
    return out


# revision 16
# speedup vs baseline: 1.1985x; 1.0320x over previous
"""DigitCaps dynamic-routing kernel for 8 Trainium2 NeuronCores (v3).

Data-parallel over batch (32/core), W replicated, b_ij kept globally
consistent via fp32 AllReduces of the [1152,10] agreement tensor (validated:
per-core batch means diverge far past tolerance, so the collectives are
semantically required).

Structure (measured engine rates drive the assignment):
- All inputs ship as bf16 in final on-chip layouts (host-side cast +
  permutation): no staging, no on-device casts, ~4.2 MB input DMA. t=0
  compute hides under the fixed ~62us collective-stream init barrier.
- Each routing update's AllReduce is SPLIT into route-chunk halves (g0-4,
  g5-8). The next iteration's softmax/fold/matmuls for the first half start
  as soon as its half lands, hiding most of the second half's latency; the
  t=0 first half also serves as the collective warmup.
- s accumulates 72 chunked matmuls into one PSUM bank; pT ships with each
  32-batch block replicated 3x so s lands replicated on partitions 0..95 and
  the squash writes the three partition-diagonal blocks of the block-diagonal
  agreement operand vb3 with lane-aligned ops.
- Agreement phase per chunk: 3 matmuls against vb3 (3 i-planes each), ACT
  drains PSUM->SBUF bf16, W-multiply split Pool(2)/DVE(1) scattered into
  (c,k,d) order, one DVE X-reduce -> A chunk, staged per-half for the
  collective. Engines pipeline across chunks.
- Wc folds are DVE-only (Pool measured 2.6x slower and concurrent DVE+Pool
  big SBUF ops degrade both ~2.4x); softmax skips max-subtraction
  (|b_ij| <= 2.8 measured); squash uses bit-magic rsqrt + 1 Newton step.
  Numpy bit-model of these quantization points: rel_max 6.0e-3.
"""
import numpy as np
import ml_dtypes
from contextlib import ExitStack

import concourse.bass as bass
from concourse import bacc
import concourse.tile as tile
from concourse import mybir
from concourse.bass_utils import run_bass_kernel_spmd

N_CORES = 8
B_FULL, R, C, D, I = 256, 1152, 10, 16, 8
B = B_FULL // N_CORES          # 32 batch per core
G = R // 128                   # 9 chunks of 128 routes
CD = C * D                     # 160
CDI = C * D * I                # 1280
NUM_IT = 3
PT_W = 72 * 96 + 32            # 6944: 72 (i,g) blocks of 3x-replicated batch
P3_W = G * 3 * 128             # 3456
N_WARM = 33                    # PE-warming dummy matmuls in the t=1 AR gap

FP32 = mybir.dt.float32
BF16 = mybir.dt.bfloat16
ALU = mybir.AluOpType
AX = mybir.AxisListType
AF = mybir.ActivationFunctionType


def _build_body(ctx: ExitStack, tc: "tile.TileContext", pt_dram, p3_dram,
                w_dram, v_dram):
    nc = tc.nc

    consts = ctx.enter_context(tc.tile_pool(name="consts", bufs=1))
    pers = ctx.enter_context(tc.tile_pool(name="pers", bufs=1))
    small = ctx.enter_context(tc.tile_pool(name="small", bufs=2))
    work = ctx.enter_context(tc.tile_pool(name="work", bufs=2))
    wcp = ctx.enter_context(tc.tile_pool(name="wcp", bufs=1))
    dram = ctx.enter_context(tc.tile_pool(name="dram", bufs=2, space="DRAM"))
    ps_s = ctx.enter_context(tc.tile_pool(name="ps_s", bufs=1, space="PSUM"))
    ps_y = ctx.enter_context(tc.tile_pool(name="ps_y", bufs=2, space="PSUM"))
    ps_w = ctx.enter_context(tc.tile_pool(name="ps_w", bufs=1, space="PSUM"))

    # ---------------- input tiles, DMA'd bf16 in final layout ----------------
    # pT[j, k*96 + rep*32 + b] = p[b, (g*128+j)*8 + i], k = i*G+g, rep 0..2
    pT = pers.tile([128, PT_W], BF16, tag="pT")
    # p3[(i_rel*32+b), (g*3+grp)*128 + j] = p[b, r, grp*3+i_rel] (i=8 zeros)
    p3 = pers.tile([96, P3_W], BF16, tag="p3")
    # W bf16, (r -> partition j within chunk g, free (i, c, d))
    wre = [pers.tile([128, CDI], BF16, tag=f"wre{g}", name=f"wre{g}")
           for g in range(G)]

    for g in (0, 1, 2, 3, 6):
        nc.sync.dma_start(wre[g][:], w_dram[128 * g:128 * (g + 1), :])
    H = PT_W // 2
    nc.scalar.dma_start(pT[:, :H], pt_dram[:, :H])
    nc.scalar.dma_start(pT[:, H:], pt_dram[:, H:])
    for g in (4, 5, 7, 8):
        nc.scalar.dma_start(wre[g][:], w_dram[128 * g:128 * (g + 1), :])
    H3 = P3_W // 2
    nc.gpsimd.dma_start(p3[:, :H3], p3_dram[:, :H3])
    nc.gpsimd.dma_start(p3[:, H3:], p3_dram[:, H3:])

    magic_t = consts.tile([96, C], mybir.dt.int32, tag="magic_t")
    nc.gpsimd.memset(magic_t[:], 0x5F3759DF)
    # routing logits, [128, (g c)] layout
    bij = pers.tile([128, G * C], FP32, tag="bij")
    nc.gpsimd.memset(bij[:], 0.0)
    # block-diagonal moving operand for the agreement matmuls; off-diagonal
    # stays zero forever, diagonal blocks are rewritten by the squash
    vb3 = pers.tile([96, 3 * CD], BF16, tag="vb3")
    nc.gpsimd.memset(vb3[:], 0.0)

    cc_out_prev = None

    def softmax_fold(t):
        """b update + softmax + Wc folds; cbd split so fold g0 starts early."""
        acc = small.tile([128, G * C], FP32, tag="acc", name=f"acc{t}")
        nc.sync.dma_start(acc[:], cc_out_prev[:])
        nc.vector.scalar_tensor_tensor(
            out=bij[:], in0=acc[:], scalar=1.0 / B_FULL,
            op0=ALU.mult, in1=bij[:], op1=ALU.add)
        eb = small.tile([128, G * C], FP32, tag="eb", name=f"eb{t}")
        nc.scalar.activation(eb[:], bij[:], AF.Exp)
        ebv = eb[:].rearrange("p (g c) -> p g c", g=G, c=C)
        sm = small.tile([128, G], FP32, tag="sm", name=f"sm{t}")
        nc.vector.tensor_reduce(sm[:], ebv, axis=AX.X, op=ALU.add)
        rc = small.tile([128, G], FP32, tag="rc", name=f"rc{t}")
        nc.vector.reciprocal(rc[:], sm[:])
        cbb = small.tile([128, G * C], BF16, tag="cbb", name=f"cbb{t}")
        cbv = cbb[:].rearrange("p (g c) -> p g c", g=G, c=C)
        rcb = rc[:].unsqueeze(2).broadcast_to([128, G, C])
        nc.vector.tensor_tensor(cbv, ebv, rcb, op=ALU.mult)
        # expand c over d (innermost stride-0) in two pieces so each fold's
        # in1 has a packed innermost dim (2x DVE mode), broadcast only over k
        cbd = small.tile([128, G * CD], BF16, tag="cbd", name=f"cbd{t}")
        for lo, hi in ((0, 3), (3, G)):
            nc.vector.tensor_copy(
                cbd[:, lo * CD:hi * CD].rearrange(
                    "p (g c d) -> p g c d", g=hi - lo, c=C, d=D),
                cbb[:, lo * C:hi * C].rearrange("p (g c) -> p g c",
                                                g=hi - lo, c=C)
                .unsqueeze(3).broadcast_to([128, hi - lo, C, D]))
        wcs = []
        for g in range(G):
            wcg = wcp.tile([128, CDI], BF16, tag=f"wc{g}", name=f"wc{g}_{t}")
            in1 = cbd[:, g * CD:(g + 1) * CD].unsqueeze(1) \
                .broadcast_to([128, I, CD])
            nc.vector.tensor_tensor(
                wcg[:].rearrange("p (k x) -> p k x", k=I, x=CD),
                wre[g][:].rearrange("p (k x) -> p k x", k=I, x=CD),
                in1, op=ALU.mult)
            wcs.append(wcg)
        return wcs

    # ---------------- routing iterations ----------------
    for t in range(NUM_IT):
        last = t == NUM_IT - 1
        if t == 0:
            wc = wre                      # c_ij uniform: fold 0.1 into squash
            e_scale = 0.01
        else:
            e_scale = 1.0

        # s[b,(c,d)] = sum_{r,i} p * Wc: 72 matmuls, one accumulating PSUM
        # bank; output replicated on partition blocks 0:32/32:64/64:96.
        # At t>0 the fold for each half runs as its AllReduce half lands.
        if t > 0:
            wc = softmax_fold(t)
        s_ps = ps_s.tile([128, CD], FP32, tag="s_ps", name=f"s_ps_{t}")
        n_tot = G * I
        n_mm = 0
        for g in range(G):
            for i in range(I):
                k = i * G + g
                nc.tensor.matmul(
                    s_ps[:],
                    pT[:, k * 96:k * 96 + 128],
                    wc[g][:, i * CD:(i + 1) * CD],
                    start=(n_mm == 0),
                    stop=(n_mm == n_tot - 1),
                )
                n_mm += 1

        # squash at [96, *]: v = s * e*sqrt(sq)/(1+e*sq), sq = sum_d s^2
        s2 = small.tile([96, CD], FP32, tag="s2", name=f"s2_{t}")
        nc.scalar.activation(s2[:], s_ps[0:96, :], AF.Square)

        sq = small.tile([96, C], FP32, tag="sq", name=f"sq_{t}")
        nc.vector.tensor_reduce(sq[:],
                                s2[:].rearrange("b (c d) -> b c d", c=C, d=D),
                                axis=AX.X, op=ALU.add)
        h32 = small.tile([96, C], mybir.dt.int32, tag="h32", name=f"h32_{t}")
        nc.vector.tensor_scalar(h32[:], sq[:].bitcast(mybir.dt.int32), 1,
                                None, op0=ALU.logical_shift_right)
        y0i = small.tile([96, C], mybir.dt.int32, tag="y0i", name=f"y0i_{t}")
        nc.vector.tensor_tensor(y0i[:], magic_t[:], h32[:], op=ALU.subtract)
        y = y0i[:].bitcast(FP32)
        ya = small.tile([96, C], FP32, tag="ya", name=f"ya_{t}")
        yb = small.tile([96, C], FP32, tag="yb", name=f"yb_{t}")
        nc.vector.tensor_tensor(ya[:], y, y, op=ALU.mult)
        nc.vector.tensor_tensor(yb[:], ya[:], sq[:], op=ALU.mult)
        nc.vector.tensor_scalar(yb[:], yb[:], -0.5, 1.5, op0=ALU.mult,
                                op1=ALU.add)
        yn = small.tile([96, C], FP32, tag="yn", name=f"yn_{t}")
        nc.vector.tensor_tensor(yn[:], y, yb[:], op=ALU.mult)
        r1 = small.tile([96, C], FP32, tag="r1", name=f"r1_{t}")
        nc.vector.tensor_tensor(r1[:], sq[:], yn[:], op=ALU.mult)
        den = small.tile([96, C], FP32, tag="den", name=f"den_{t}")
        nc.vector.tensor_scalar(den[:], sq[:], e_scale, 1.0, op0=ALU.mult,
                                op1=ALU.add)
        rec = small.tile([96, C], FP32, tag="rec", name=f"rec_{t}")
        nc.vector.reciprocal(rec[:], den[:])
        fac = small.tile([96, C], FP32, tag="fac", name=f"fac_{t}")
        nc.vector.tensor_tensor(fac[:], r1[:], rec[:], op=ALU.mult)

        if last:
            v32 = small.tile([B, CD], FP32, tag="v32")
            fb = fac[0:B, :].unsqueeze(2).broadcast_to([B, C, D])
            nc.vector.scalar_tensor_tensor(
                out=v32[:].rearrange("b (c d) -> b c d", c=C, d=D),
                in0=s_ps[0:B, :].rearrange("b (c d) -> b c d", c=C, d=D),
                scalar=e_scale, op0=ALU.mult, in1=fb, op1=ALU.mult)
            nc.sync.dma_start(v_dram[:, :], v32[:])
            continue

        # diagonal blocks of vb3 (s_ps replication keeps this lane-aligned)
        for rp in range(3):
            pa, pb_ = rp * 32, (rp + 1) * 32
            fb = fac[pa:pb_, :].unsqueeze(2).broadcast_to([32, C, D])
            nc.vector.scalar_tensor_tensor(
                out=vb3[pa:pb_, rp * CD:(rp + 1) * CD]
                    .rearrange("b (c d) -> b c d", c=C, d=D),
                in0=s_ps[pa:pb_, :].rearrange("b (c d) -> b c d", c=C, d=D),
                scalar=e_scale, op0=ALU.mult, in1=fb, op1=ALU.mult)

        # ---- agreement: A[r,c] = sum_{i,d} W . (p^T v), one AllReduce ----
        Apart = pers.tile([128, G * C], FP32, tag="Apart", name=f"Apart{t}")
        cc_in = dram.tile([128, G * C], FP32, tag="cc_in", name=f"cc_in{t}")
        for g in range(G):
            y0 = ps_y.tile([128, 3 * CD], FP32, tag="y0", name=f"y0_{g}_{t}")
            y1 = ps_y.tile([128, 3 * CD], FP32, tag="y1", name=f"y1_{g}_{t}")
            y2 = ps_y.tile([128, 2 * CD], FP32, tag="y2", name=f"y2_{g}_{t}")
            c0 = (3 * g) * 128
            nc.tensor.matmul(y0[:], p3[:, c0:c0 + 128], vb3[:],
                             start=True, stop=True)
            nc.tensor.matmul(y1[:], p3[:, c0 + 128:c0 + 256], vb3[:],
                             start=True, stop=True)
            nc.tensor.matmul(y2[:], p3[0:64, c0 + 256:c0 + 384],
                             vb3[0:64, 0:2 * CD], start=True, stop=True)
            y0sb = work.tile([128, 3 * CD], BF16, tag="y0sb",
                             name=f"y0sb{g}_{t}")
            y1sb = work.tile([128, 3 * CD], BF16, tag="y1sb",
                             name=f"y1sb{g}_{t}")
            y2sb = work.tile([128, 2 * CD], BF16, tag="y2sb",
                             name=f"y2sb{g}_{t}")
            nc.scalar.copy(y0sb[:], y0[:])
            nc.scalar.copy(y1sb[:], y1[:])
            nc.scalar.copy(y2sb[:], y2[:])
            # prod in (c, k, d) order so one X-reduce yields A[:, (g c)]
            prod = work.tile([128, CDI], BF16, tag="prod",
                             name=f"prod{g}_{t}")
            pv = prod[:].rearrange("p (c k d) -> p k c d", c=C, k=I, d=D)
            wv = wre[g][:].rearrange("p (k c d) -> p k c d", k=I, c=C, d=D)
            nc.gpsimd.tensor_tensor(
                pv[:, 0:3], wv[:, 0:3],
                y0sb[:].rearrange("p (k c d) -> p k c d", k=3, c=C, d=D),
                op=ALU.mult)
            nc.vector.tensor_tensor(
                pv[:, 3:6], wv[:, 3:6],
                y1sb[:].rearrange("p (k c d) -> p k c d", k=3, c=C, d=D),
                op=ALU.mult)
            nc.gpsimd.tensor_tensor(
                pv[:, 6:8], wv[:, 6:8],
                y2sb[:].rearrange("p (k c d) -> p k c d", k=2, c=C, d=D),
                op=ALU.mult)
            nc.vector.tensor_reduce(
                Apart[:, g * C:(g + 1) * C],
                prod[:].rearrange("p (c x) -> p c x", c=C, x=I * D),
                axis=AX.X, op=ALU.add)
            nc.sync.dma_start(cc_in[:, g * C:(g + 1) * C],
                              Apart[:, g * C:(g + 1) * C])
        cc_out_prev = dram.tile([128, G * C], FP32, tag="cc_out",
                                name=f"cc_out{t}", addr_space="Shared")
        nc.gpsimd.collective_compute(
            "AllReduce", ALU.add,
            replica_groups=[list(range(N_CORES))],
            ins=[cc_in[:].opt()], outs=[cc_out_prev[:].opt()])

        if t == 1:
            # PE stays clocked at 1.2 GHz unless kept busy (~3.4us HAM
            # windows). Fill the AllReduce gap before the final iteration
            # with dummy matmuls chained on the last prod tile so the t=2
            # s-matmuls run at 2.4 GHz. Results are never read.
            warm_ps = ps_w.tile([128, 512], FP32, tag="warm_ps")
            for w in range(N_WARM):
                nc.tensor.matmul(warm_ps[:], prod[:, 0:128], prod[:, 0:512],
                                 start=True, stop=True)


_CACHED = None


def _build():
    global _CACHED
    if _CACHED is not None:
        return _CACHED
    nc = bacc.Bacc("TRN2", target_bir_lowering=False, debug=False,
                   num_devices=N_CORES)
    pt_dram = nc.dram_tensor("pt_in", [128, PT_W], BF16,
                             kind="ExternalInput").ap()
    p3_dram = nc.dram_tensor("p3_in", [96, P3_W], BF16,
                             kind="ExternalInput").ap()
    w_dram = nc.dram_tensor("w_in", [R, CDI], BF16, kind="ExternalInput").ap()
    v_dram = nc.dram_tensor("v_out", [B, CD], FP32, kind="ExternalOutput").ap()
    with tile.TileContext(nc) as tc:
        with ExitStack() as ctx:
            _build_body(ctx, tc, pt_dram, p3_dram, w_dram, v_dram)
    nc.finalize()
    _CACHED = nc
    return nc


def kernel(prim_caps: np.ndarray, W: np.ndarray, _trace: bool = False):
    assert prim_caps.shape == (B_FULL, R, I) and W.shape == (1, R, C, D, I)
    nc = _build()
    bf16 = ml_dtypes.bfloat16
    w_flat = np.ascontiguousarray(
        W.reshape(R, C, D, I).transpose(0, 3, 1, 2).reshape(R, CDI)
        .astype(bf16))
    p32 = prim_caps.astype(np.float32)
    in_maps = []
    for k in range(N_CORES):
        pk = p32[k * B:(k + 1) * B]
        pk4 = pk.reshape(B, G, 128, I)
        ptk = np.zeros((128, PT_W), np.float32)
        ptk[:, :72 * 96] = np.broadcast_to(
            pk4.transpose(2, 3, 1, 0)[:, :, :, None, :],
            (128, I, G, 3, B)).reshape(128, 72 * 96)
        p9 = np.zeros((B, G, 128, 9), np.float32)
        p9[..., :I] = pk4
        p3k = p9.reshape(B, G, 128, 3, 3).transpose(4, 0, 1, 3, 2) \
            .reshape(96, P3_W)
        in_maps.append({"pt_in": ptk.astype(bf16),
                        "p3_in": np.ascontiguousarray(p3k.astype(bf16)),
                        "w_in": w_flat})
    res = run_bass_kernel_spmd(nc, in_maps, core_ids=list(range(N_CORES)),
                               trace=_trace)
    out = np.concatenate(
        [res.results[k]["v_out"].reshape(B, C, D, 1) for k in range(N_CORES)],
        axis=0)
    if _trace:
        return out, res
    return out
